# revision 1
# baseline (speedup 1.0000x reference)
"""CascadeHadamardLinear Trainium2 kernel (8-core data-parallel over tokens).

Math per token row x[4096]:
  x_rot = (x * S_in) @ blockdiag(H_128)     (fp32-exact on PE)
  x_q   = NVFP4 fake-quant of x_rot (16-elem blocks, e2m1 snap, RNE)
  out   = x_q @ W^T + (x_rot @ la^T) @ lb^T + bias

Host side: shard 8192 tokens over 8 cores (1024 each); pre-transpose
x/W/la/lb (xT [4096,1024] fp32, wT [4096,4096] bf16, laT, lbT) so every
matmul operand arrives with the contraction dim on partitions.

Device, per core, tokens processed in 2 halves for phase overlap:
  P1(h): per 4-block group: rotation MMs (lhsT = xT block, rhs = S-folded
    H_mod, fp32) -> PSUM [128t, 512]; quant: absmax-16 reduce (DVE),
    recip, z = x*(6/amax), t2 = RNE_int(z) (fused tensor_scalar), custom
    DVE ops SNAP_SEL23/SNAP_SELF (branch-free e2m1 snap via 1.5*2^k
    magic-add rounding + select on z^2), xq = f*(amax/6) -> bf16
    (GPSIMD); LoRA1 t1 = la_eff^T @ x accumulates in PSUM (la_eff =
    diag(S) Hbd la^T, precomputed on device); PE-transpose xq -> xqT.
  P2(h): out[t,o] accumulation chain in PSUM: bias (K=1 ones MM) +
    sum_dblk xqT^T @ wT + t1T^T @ lbT, ACT evac fp32, DMA out.
Emission order p1(0), p1(1), p2(0), p2(1): the Tile scheduler backfills
PE idle slots of the DVE-bound P1(1) with ready P2(0) matmuls.
"""

import os
import sys

for _p in ("/opt/trn_rl_repo",):
    if os.path.isdir(_p) and _p not in sys.path:
        sys.path.insert(0, _p)

import numpy as np

import concourse.bass as bass
import concourse.mybir as mybir
import concourse.tile as tile
from concourse import bacc
from concourse.bass_utils import run_bass_kernel_spmd

F32 = np.float32

# ---------------- problem constants (hardcoded per contract) ----------------
B, S, D_IN, D_OUT, RANK, HBS = 4, 2048, 4096, 4096, 32, 128
NTOK = B * S                  # 8192
NCORES = 8
NT = NTOK // NCORES           # 1024 tokens per core
NJ = D_IN // HBS              # 32 hadamard blocks
QB = 16                       # quant block size
TS_N = NT // 128              # 8 token sub-tiles of 128
OG_N = D_OUT // 512           # 8 output column groups

# quant snap constants (1.5*2^k magic so ulp is uniform on both sides of c)
C_INT = 12582912.0            # 1.5*2^23, ulp 1.0
C_HALF = 6291456.0            # 1.5*2^22, ulp 0.5
TH23 = 20.25                  # 4.5^2
THF = 5.0625                  # 2.25^2

# ---------------- custom DVE ops (e2m1 level snap) ----------------
def _register_snap_ops():
    from concourse.dve_spec import (
        Spec, Src0, Src1, C0, C1, lower as dve_lower, sq, select, _has_src1,
    )
    from concourse.dve_ops import (
        DveOp, OPS, CUSTOM_DVE_SPECS, _SUB_OPCODE_FOR_NAME, _CUSTOM_DVE_ROW_BASE,
    )
    from concourse.dve_uop import DveOpSpec
    from concourse.dve_table_gen import dve_ver_for

    def _ref_sel23(in0, in1, c0, c1, c2):
        z = in0.astype(F32)
        c3 = F32(2.0) * F32(c1)
        t3 = (z + c3) - c3
        return np.where(z * z < F32(c0), in1.astype(F32), t3).astype(F32)

    def _ref_self(in0, in1, c0, c1, c2):
        z = in0.astype(F32)
        c1 = F32(c1)
        t1 = (z + c1) - c1
        return np.where(z * z < F32(c0), t1, in1.astype(F32)).astype(F32)

    def _mk(name, body, ref):
        if name in _SUB_OPCODE_FOR_NAME:
            return next(op for op in OPS if op.name == name)
        spec = Spec(body=body, reference=ref)
        row = _CUSTOM_DVE_ROW_BASE + len(OPS)
        assert row < 0x20
        ver = dve_ver_for("TRN2")
        uops = dve_lower(spec, ver=ver)
        sha = DveOpSpec(
            name=name, opcode=row, uops=uops, rd1_en=_has_src1(spec)
        ).sha(ver)
        op = DveOp(name, spec, subdim=False, uops_sha={ver: sha})
        OPS.append(op)
        CUSTOM_DVE_SPECS[name] = spec
        _SUB_OPCODE_FOR_NAME[name] = row
        return op

    z = Src0
    c3 = C1 + C1
    sel23 = _mk(
        "SNAP_SEL23_ANT",
        select(sq(z) < C0, Src1, (z + c3) - c3),
        _ref_sel23,
    )
    self_ = _mk(
        "SNAP_SELF_ANT",
        select(sq(z) < C0, (z + C1) - C1, Src1),
        _ref_self,
    )
    return sel23, self_


SNAP_SEL23, SNAP_SELF = _register_snap_ops()


# ---------------- device kernel ----------------
def _build_nc():
    nc = bacc.Bacc(
        "TRN2", target_bir_lowering=False, debug=False, num_devices=NCORES
    )
    dt = mybir.dt
    xT = nc.dram_tensor("xT", [D_IN, NT], dt.float32, kind="ExternalInput")
    wT = nc.dram_tensor("wT", [D_IN, D_OUT], dt.bfloat16, kind="ExternalInput")
    H = nc.dram_tensor("H", [HBS, HBS], dt.float32, kind="ExternalInput")
    Scol = nc.dram_tensor("Scol", [HBS, NJ], dt.float32, kind="ExternalInput")
    laT = nc.dram_tensor("laT", [D_IN, RANK], dt.float32, kind="ExternalInput")
    lbT = nc.dram_tensor("lbT", [RANK, D_OUT], dt.bfloat16, kind="ExternalInput")
    bias = nc.dram_tensor("bias", [1, D_OUT], dt.bfloat16, kind="ExternalInput")
    y = nc.dram_tensor("y", [NT, D_OUT], dt.float32, kind="ExternalOutput")

    with tile.TileContext(nc) as tc:
        _emit(nc, tc, xT, wT, H, Scol, laT, lbT, bias, y)
    nc.compile()
    return nc


def _emit(nc, tc, xT, wT, H, Scol, laT, lbT, bias, y):
    from contextlib import ExitStack

    dt = mybir.dt
    Alu = mybir.AluOpType
    Act = mybir.ActivationFunctionType
    HT = NT // 2          # tokens per half (512)
    TS_H = HT // 128      # 4 token sub-tiles per half

    with ExitStack() as ctx:
        singles = ctx.enter_context(tc.tile_pool(name="singles", bufs=1))
        consts1 = ctx.enter_context(tc.tile_pool(name="consts1", bufs=1))
        xqT_pool = ctx.enter_context(tc.tile_pool(name="xqT", bufs=1))

        ones_bf = singles.tile([1, 128], dt.bfloat16)
        nc.vector.memset(ones_bf[:], 1.0)
        t1T_bf = singles.tile([RANK, NT], dt.bfloat16)

        H_sb = consts1.tile([HBS, HBS], dt.float32)
        nc.sync.dma_start(out=H_sb[:], in_=H[:])
        S_sb = consts1.tile([HBS, NJ], dt.float32)
        nc.sync.dma_start(out=S_sb[:], in_=Scol[:])
        H_mod = consts1.tile([HBS, NJ, HBS], dt.float32)
        for j in range(NJ):
            nc.vector.tensor_scalar(
                out=H_mod[:, j, :], in0=H_sb[:], scalar1=S_sb[:, j : j + 1],
                scalar2=None, op0=Alu.mult,
            )
        la_eff_bf = consts1.tile([HBS, NJ, RANK], dt.bfloat16)
        ident_bf = consts1.tile([128, 128], dt.bfloat16)
        from concourse.masks import make_identity
        make_identity(nc, ident_bf[:])

        with tc.tile_pool(name="setup_tmp", bufs=1) as stp:
            laT_sb = stp.tile([HBS, NJ, RANK], dt.float32)
            nc.sync.dma_start(
                out=laT_sb[:], in_=laT[:].rearrange("(j c) r -> c j r", c=HBS)
            )
            la_eff = stp.tile([HBS, NJ, RANK], dt.float32)
            with tc.tile_pool(name="psla", bufs=2, space="PSUM") as psla_pool:
                for j in range(NJ):
                    ps = psla_pool.tile([HBS, RANK], dt.float32)
                    nc.tensor.matmul(ps[:], lhsT=H_sb[:], rhs=laT_sb[:, j, :])
                    nc.scalar.activation(
                        out=la_eff[:, j, :], in_=ps[:], func=Act.Copy,
                        scale=S_sb[:, j : j + 1],
                    )
            nc.scalar.copy(out=la_eff_bf[:], in_=la_eff[:])

        # xqT[c, dblk, t] : feature-major quantized activations (bf16)
        xqT = xqT_pool.tile([128, NJ, NT], dt.bfloat16)

        # working pools (live across both halves)
        xt_pool = ctx.enter_context(tc.tile_pool(name="xt", bufs=5))
        xtbf_pool = ctx.enter_context(tc.tile_pool(name="xtbf", bufs=5))
        qtmp = ctx.enter_context(tc.tile_pool(name="qtmp", bufs=6))
        qsm = ctx.enter_context(tc.tile_pool(name="qsm", bufs=4))
        xq_pool = ctx.enter_context(tc.tile_pool(name="xq", bufs=6))
        smo = ctx.enter_context(tc.tile_pool(name="smo", bufs=2))
        wbf_pool = ctx.enter_context(tc.tile_pool(name="wbf", bufs=2))
        out_pool = ctx.enter_context(tc.tile_pool(name="out", bufs=4))
        rot_ps = ctx.enter_context(tc.tile_pool(name="rotps", bufs=3, space="PSUM"))
        tr_ps = ctx.enter_context(tc.tile_pool(name="trps", bufs=1, space="PSUM"))
        t1_ps = ctx.enter_context(tc.tile_pool(name="t1ps", bufs=2, space="PSUM"))
        out_ps = ctx.enter_context(tc.tile_pool(name="outps", bufs=2, space="PSUM"))

        def emit_p1(h, t0, tlen):
            hsl = slice(t0, t0 + tlen)
            TS_H = tlen // 128
            # LoRA1 accumulators in <=512-token chunks (one PSUM bank each)
            lchunks = []
            o = 0
            while o < tlen:
                w = min(512, tlen - o)
                acc = t1_ps.tile([RANK, w], dt.float32,
                                 name=f"t1acc{h}_{o}", tag="t1acc")
                lchunks.append((o, w, acc))
                o += w
            for jg in range(NJ // 4):
                xts = []
                for dj in range(4):
                    j = 4 * jg + dj
                    xt = xt_pool.tile([HBS, tlen], dt.float32, name=f"xt{h}_{j}", tag="xt")
                    nc.sync.dma_start(out=xt[:], in_=xT[j * HBS : (j + 1) * HBS, hsl])
                    xts.append(xt)
                    xt_bf = xtbf_pool.tile([HBS, tlen], dt.bfloat16, name=f"xtbf{h}_{j}", tag="xtbf")
                    nc.scalar.copy(out=xt_bf[:], in_=xt[:])
                    for (o, w, acc) in lchunks:
                        nc.tensor.matmul(
                            acc[:], lhsT=la_eff_bf[:, j, :],
                            rhs=xt_bf[:, o : o + w],
                            start=(j == 0), stop=(j == NJ - 1),
                        )
                xq_tiles = []
                for ts in range(TS_H):
                    bank = rot_ps.tile([128, 512], dt.float32, name=f"bank{h}_{jg}_{ts}", tag="bank")
                    for dj in range(4):
                        j = 4 * jg + dj
                        nc.tensor.matmul(
                            bank[:, dj * HBS : (dj + 1) * HBS],
                            lhsT=xts[dj][:, ts * 128 : (ts + 1) * 128],
                            rhs=H_mod[:, j, :],
                            start=(dj == 0), stop=(dj == 3),
                        )
                    nb = 512 // QB
                    amax = qsm.tile([128, nb], dt.float32, name=f"amax{h}{jg}{ts}", tag="amax")
                    nc.vector.tensor_reduce(
                        out=amax[:], in_=bank[:].rearrange("p (b s) -> p b s", s=QB),
                        axis=mybir.AxisListType.X, op=Alu.max,
                        apply_absolute_value=True,
                    )
                    ra = qsm.tile([128, nb], dt.float32, name=f"ra{h}{jg}{ts}", tag="ra")
                    nc.vector.reciprocal(out=ra[:], in_=amax[:])
                    rs6 = qsm.tile([128, nb], dt.float32, name=f"rs6{h}{jg}{ts}", tag="rs6")
                    nc.scalar.mul(out=rs6[:], in_=ra[:], mul=6.0)
                    sc = qsm.tile([128, nb], dt.float32, name=f"sc{h}{jg}{ts}", tag="sc")
                    nc.scalar.mul(out=sc[:], in_=amax[:], mul=1.0 / 6.0)
                    z = qtmp.tile([128, 512], dt.float32, name=f"z{h}{jg}{ts}", tag="qt")
                    nc.vector.tensor_tensor(
                        out=z[:].rearrange("p (b s) -> p b s", s=QB),
                        in0=bank[:].rearrange("p (b s) -> p b s", s=QB),
                        in1=rs6[:].unsqueeze(2).broadcast_to([128, nb, QB]),
                        op=Alu.mult,
                    )
                    t2 = qtmp.tile([128, 512], dt.float32, name=f"t2{h}{jg}{ts}", tag="qt")
                    nc.vector.tensor_scalar(
                        out=t2[:], in0=z[:], scalar1=C_INT, scalar2=C_INT,
                        op0=Alu.add, op1=Alu.subtract,
                    )
                    r23 = qtmp.tile([128, 512], dt.float32, name=f"r23{h}{jg}{ts}", tag="qt")
                    nc.vector._custom_dve(
                        SNAP_SEL23, out=r23[:], in0=z[:], in1=t2[:], s0=TH23, s1=C_INT,
                    )
                    f = qtmp.tile([128, 512], dt.float32, name=f"f{h}{jg}{ts}", tag="qt")
                    nc.vector._custom_dve(
                        SNAP_SELF, out=f[:], in0=z[:], in1=r23[:], s0=THF, s1=C_HALF,
                    )
                    xq_t = xq_pool.tile([128, 512], dt.bfloat16, name=f"xq{h}{jg}{ts}", tag="xq")
                    nc.gpsimd.tensor_tensor(
                        out=xq_t[:].rearrange("p (b s) -> p b s", s=QB),
                        in0=f[:].rearrange("p (b s) -> p b s", s=QB),
                        in1=sc[:].unsqueeze(2).broadcast_to([128, nb, QB]),
                        op=Alu.mult,
                    )
                    xq_tiles.append(xq_t)
                for dj in range(4):
                    j = 4 * jg + dj
                    pt = tr_ps.tile([128, tlen], dt.bfloat16, name=f"pt{h}_{j}", tag="pt")
                    for ts in range(TS_H):
                        nc.tensor.matmul(
                            pt[:, ts * 128 : (ts + 1) * 128],
                            lhsT=xq_tiles[ts][:, dj * HBS : (dj + 1) * HBS],
                            rhs=ident_bf[:], is_transpose=True,
                            start=(ts == 0), stop=(ts == TS_H - 1),
                        )
                    nc.scalar.copy(out=xqT[:, j, hsl], in_=pt[:])
            for (o, w, acc) in lchunks:
                nc.scalar.copy(out=t1T_bf[:, t0 + o : t0 + o + w], in_=acc[:])

        def emit_p2(h, t0, tlen):
            TS_H = tlen // 128
            for og in range(OG_N):
                osl = slice(og * 512, (og + 1) * 512)
                wbf = wbf_pool.tile([128, NJ, 512], dt.bfloat16, name=f"wbf{h}_{og}", tag="wbf")
                nc.sync.dma_start(
                    out=wbf[:], in_=wT[:, osl].rearrange("(j c) o -> c j o", c=HBS)
                )
                bias_og = smo.tile([1, 512], dt.bfloat16, name=f"biaso{h}{og}", tag="biaso")
                nc.sync.dma_start(out=bias_og[:], in_=bias[:, osl])
                lb_og = smo.tile([RANK, 512], dt.bfloat16, name=f"lbo{h}{og}", tag="lbo")
                nc.sync.dma_start(out=lb_og[:], in_=lbT[:, osl])
                for th in range(TS_H):
                    tsl = slice(t0 + th * 128, t0 + (th + 1) * 128)
                    po = out_ps.tile([128, 512], dt.float32, name=f"po{h}{og}{th}", tag="po")
                    nc.tensor.matmul(
                        po[:], lhsT=ones_bf[:], rhs=bias_og[:],
                        start=True, stop=False,
                    )
                    for dblk in range(NJ):
                        nc.tensor.matmul(
                            po[:], lhsT=xqT[:, dblk, tsl], rhs=wbf[:, dblk, :],
                            start=False, stop=False,
                        )
                    nc.tensor.matmul(
                        po[:], lhsT=t1T_bf[:, tsl], rhs=lb_og[:],
                        start=False, stop=True,
                    )
                    ot = out_pool.tile([128, 512], dt.float32, name=f"ot{h}{og}{th}", tag="ot")
                    nc.scalar.copy(out=ot[:], in_=po[:])
                    nc.sync.dma_start(out=y[tsl, osl], in_=ot[:])

        emit_p1(0, 0, 256)
        emit_p1(1, 256, NT - 256)
        emit_p2(0, 0, 256)
        emit_p2(1, 256, NT - 256)


_NC_CACHE = None


def _get_nc():
    global _NC_CACHE
    if _NC_CACHE is None:
        _NC_CACHE = _build_nc()
    return _NC_CACHE


# ---------------- host wrapper ----------------
def kernel(x, S_in, H_block, w_quantized, lora_a, lora_b, bias):
    x = np.asarray(x, dtype=F32)
    S_in = np.asarray(S_in, dtype=F32)
    H_block = np.ascontiguousarray(np.asarray(H_block, dtype=F32))
    w_quantized = np.asarray(w_quantized, dtype=F32)
    lora_a = np.asarray(lora_a, dtype=F32)
    lora_b = np.asarray(lora_b, dtype=F32)
    bias = np.asarray(bias, dtype=F32)

    import ml_dtypes
    BF16 = ml_dtypes.bfloat16
    x_flat = x.reshape(NTOK, D_IN)
    wT = np.ascontiguousarray(w_quantized.T.astype(BF16))   # [D_IN, D_OUT] bf16
    laT = np.ascontiguousarray(lora_a.T)                    # [D_IN, RANK]
    lbT = np.ascontiguousarray(lora_b.T.astype(BF16))       # [RANK, D_OUT] bf16
    Scol = np.ascontiguousarray(S_in.reshape(NJ, HBS).T)    # [HBS, NJ]
    bias2d = np.ascontiguousarray(bias.reshape(1, D_OUT).astype(BF16))

    nc = _get_nc()
    in_maps = []
    for c in range(NCORES):
        xT_c = np.ascontiguousarray(x_flat[c * NT : (c + 1) * NT].T)
        in_maps.append(
            {
                "xT": xT_c,
                "wT": wT,
                "H": H_block,
                "Scol": Scol,
                "laT": laT,
                "lbT": lbT,
                "bias": bias2d,
            }
        )
    res = run_bass_kernel_spmd(nc, in_maps, core_ids=list(range(NCORES)))
    out = np.concatenate([res.results[c]["y"] for c in range(NCORES)], axis=0)
    return out.reshape(B, S, D_OUT).astype(F32)



# revision 11
# speedup vs baseline: 1.0766x; 1.0766x over previous
"""CascadeHadamardLinear Trainium2 kernel (8-core data-parallel over tokens).

Math per token row x[4096]:
  x_rot = (x * S_in) @ blockdiag(H_128)     H = sign/sqrt(128) Hadamard
  x_q   = NVFP4 fake-quant of x_rot (16-elem blocks, e2m1 snap, RNE)
  out   = x_q @ W^T + (x_rot @ la^T) @ lb^T + bias

Device computes with x_rot~ = sqrt(128)*x_rot via an EXACT +/-1 sign
matrix (S folded in) in bf16, with x split host-side into x_hi + x_lo
(dual bf16) for fp32-class rotation at bf16 matmul rate. The quantizer
is scale-invariant (z = 6*v/amax), so f is unchanged; sqrt(128) is
folded out of the weights on the host (w~ = w/sqrt(128)).

Per core (1024 tokens, halves of 256/768 for phase overlap):
  p1(h): per jg (4 hadamard blocks): DMA x_hi/x_lo; LoRA1 t1 += la_eff^T
    @ x_hi (la_eff host-precomputed, includes S, H, 1/sqrt(128)); per
    128-token tile: 8 bf16 rotation MMs -> PSUM bank [128,512]; quant:
    absmax-16 (DVE), reciprocal (DVE), rs6/sc (ACT muls), fused
    z*rs6 + 3-way e2m1 magic-add snap in ONE custom DVE op, xq = f*sc
    -> bf16 (GPSIMD); PE-transpose xq -> xqT[d-major].
  p2(h): per og (512 out cols): stream w~ (bf16, host layout
    [c,og,j,o] = 32KB contiguous lines); per 128-token tile: PSUM
    accumulation of 32 main MMs + 1 merged (LoRA2+bias) K=33 MM
    (t1T has a ones row, lbT an appended bias row); ACT evac, DMA out.
Emission p1(0), w-prefetch, p2(0), p1(1), p2(1): Tile backfills PE
idle in DVE-bound p1 windows with ready p2 matmuls.
"""

import os
import sys

for _p in ("/opt/trn_rl_repo",):
    if os.path.isdir(_p) and _p not in sys.path:
        sys.path.insert(0, _p)

import numpy as np

import concourse.bass as bass
import concourse.mybir as mybir
import concourse.tile as tile
from concourse import bacc
from concourse.bass_utils import run_bass_kernel_spmd

F32 = np.float32

# ---------------- problem constants (hardcoded per contract) ----------------
B, S, D_IN, D_OUT, RANK, HBS = 4, 2048, 4096, 4096, 32, 128
NTOK = B * S                  # 8192
NCORES = 8
NT = NTOK // NCORES           # 1024 tokens per core
NJ = D_IN // HBS              # 32 hadamard blocks
QB = 16                       # quant block size
OG_N = D_OUT // 512           # 8 output column groups
H0 = 256                      # prologue half (tokens)

# quant snap constants (1.5*2^k magic so ulp is uniform on both sides of c)
C_HALF = 6291456.0            # 1.5*2^22, ulp 0.5
C_INT = 12582912.0            # 1.5*2^23, ulp 1.0
C_EVEN = 25165824.0           # 1.5*2^24, ulp 2.0
THF = 5.0625                  # 2.25^2
TH23 = 20.25                  # 4.5^2


# ---------------- custom DVE ops (e2m1 level snap, 2 passes, no t2) --------
def _register_snap_ops():
    from concourse.dve_spec import (
        Spec, Src0, Src1, C0, C1, C2, lower as dve_lower, sq, select, _has_src1,
    )
    from concourse.dve_ops import (
        DveOp, OPS, CUSTOM_DVE_SPECS, _SUB_OPCODE_FOR_NAME, _CUSTOM_DVE_ROW_BASE,
    )
    from concourse.dve_uop import DveOpSpec
    from concourse.dve_table_gen import dve_ver_for

    def _ref_a(in0, in1, c0, c1, c2):
        z = in0.astype(F32)
        r_int = (z + F32(c1)) - F32(c1)
        r_even = (z + F32(c2)) - F32(c2)
        return np.where(z * z < F32(c0), r_int, r_even).astype(F32)

    def _ref_b(in0, in1, c0, c1, c2):
        z = in0.astype(F32)
        r_half = (z + F32(c1)) - F32(c1)
        return np.where(z * z < F32(c0), r_half, in1.astype(F32)).astype(F32)

    def _mk(name, body, ref):
        if name in _SUB_OPCODE_FOR_NAME:
            return next(op for op in OPS if op.name == name)
        spec = Spec(body=body, reference=ref)
        row = _CUSTOM_DVE_ROW_BASE + len(OPS)
        assert row < 0x20
        ver = dve_ver_for("TRN2")
        uops = dve_lower(spec, ver=ver)
        sha = DveOpSpec(
            name=name, opcode=row, uops=uops, rd1_en=_has_src1(spec)
        ).sha(ver)
        op = DveOp(name, spec, subdim=False, uops_sha={ver: sha})
        OPS.append(op)
        CUSTOM_DVE_SPECS[name] = spec
        _SUB_OPCODE_FOR_NAME[name] = row
        return op

    z = Src0
    snap_a = _mk(
        "SNAP_A_ANT",
        select(sq(z) < C0, (z + C1) - C1, (z + C2) - C2),
        _ref_a,
    )
    snap_b = _mk(
        "SNAP_B_ANT",
        select(sq(z) < C0, (z + C1) - C1, Src1),
        _ref_b,
    )
    return snap_a, snap_b


SNAP_A, SNAP_B = _register_snap_ops()


# ---------------- device kernel ----------------
def _build_nc():
    nc = bacc.Bacc(
        "TRN2", target_bir_lowering=False, debug=False, num_devices=NCORES
    )
    dt = mybir.dt
    xhi = nc.dram_tensor("xhi", [HBS, NJ, NT], dt.bfloat16, kind="ExternalInput")
    xlo = nc.dram_tensor("xlo", [HBS, NJ, NT], dt.bfloat16, kind="ExternalInput")
    Hs = nc.dram_tensor("Hs", [HBS, NJ, HBS], dt.bfloat16, kind="ExternalInput")
    laE = nc.dram_tensor("laE", [HBS, NJ, RANK], dt.bfloat16, kind="ExternalInput")
    w4 = nc.dram_tensor("w4", [HBS, OG_N, NJ, 512], dt.bfloat16, kind="ExternalInput")
    lbA = nc.dram_tensor("lbA", [RANK + 1, OG_N, 512], dt.bfloat16, kind="ExternalInput")
    y = nc.dram_tensor("y", [NT, D_OUT], dt.float32, kind="ExternalOutput")

    with tile.TileContext(nc) as tc:
        _emit(nc, tc, xhi, xlo, Hs, laE, w4, lbA, y)
    nc.compile()
    return nc


def _emit(nc, tc, xhi, xlo, Hs, laE, w4, lbA, y):
    from contextlib import ExitStack

    dt = mybir.dt
    Alu = mybir.AluOpType
    Act = mybir.ActivationFunctionType

    with ExitStack() as ctx:
        consts = ctx.enter_context(tc.tile_pool(name="consts", bufs=1))

        Hs_sb = consts.tile([HBS, NJ, HBS], dt.bfloat16)
        nc.sync.dma_start(out=Hs_sb[:], in_=Hs[:])
        laE_sb = consts.tile([HBS, NJ, RANK], dt.bfloat16)
        nc.sync.dma_start(out=laE_sb[:], in_=laE[:])
        lbA_sb = consts.tile([RANK + 1, OG_N, 512], dt.bfloat16)
        nc.sync.dma_start(out=lbA_sb[:], in_=lbA[:])
        ident_bf = consts.tile([128, 128], dt.bfloat16)
        from concourse.masks import make_identity
        make_identity(nc, ident_bf[:])

        t1T = consts.tile([RANK + 1, NT], dt.bfloat16)
        nc.vector.memset(t1T[RANK : RANK + 1, :], 1.0)

        # xqT[c, j, t] : feature-major quantized activations (bf16)
        xqT = consts.tile([HBS, NJ, NT], dt.bfloat16)

        # working pools
        xh_pool = ctx.enter_context(tc.tile_pool(name="xh", bufs=2))
        xl_pool = ctx.enter_context(tc.tile_pool(name="xl", bufs=2))
        qsm = ctx.enter_context(tc.tile_pool(name="qsm", bufs=4))
        z_pool = ctx.enter_context(tc.tile_pool(name="z", bufs=3))
        r23_pool = ctx.enter_context(tc.tile_pool(name="r23", bufs=2))
        f_pool = ctx.enter_context(tc.tile_pool(name="f", bufs=3))
        xq_pool = ctx.enter_context(tc.tile_pool(name="xq", bufs=8))
        wbf_pool = ctx.enter_context(tc.tile_pool(name="wbf", bufs=2))
        out_pool = ctx.enter_context(tc.tile_pool(name="out", bufs=3))
        rot_ps = ctx.enter_context(tc.tile_pool(name="rotps", bufs=2, space="PSUM"))
        tr_ps = ctx.enter_context(tc.tile_pool(name="trps", bufs=2, space="PSUM"))
        t1_ps = ctx.enter_context(tc.tile_pool(name="t1ps", bufs=2, space="PSUM"))
        out_ps = ctx.enter_context(tc.tile_pool(name="outps", bufs=2, space="PSUM"))

        def emit_w_loads(h):
            tiles = []
            for og in range(OG_N):
                wt = wbf_pool.tile([HBS, NJ, 512], dt.bfloat16,
                                   name=f"w{h}_{og}", tag="wbf")
                nc.sync.dma_start(out=wt[:], in_=w4[:, og])
                tiles.append(wt)
            return tiles

        def emit_p1(h, t0, tlen):
            hsl = slice(t0, t0 + tlen)
            ts_n = tlen // 128
            # LoRA1 accumulators in <=512-token chunks (one PSUM bank each)
            lchunks = []
            o = 0
            while o < tlen:
                w = min(512, tlen - o)
                acc = t1_ps.tile([RANK, w], dt.float32,
                                 name=f"t1acc{h}_{o}", tag="t1acc")
                lchunks.append((o, w, acc))
                o += w
            for jg in range(NJ // 4):
                xh = xh_pool.tile([HBS, 4, tlen], dt.bfloat16,
                                  name=f"xh{h}_{jg}", tag="xh")
                nc.sync.dma_start(out=xh[:], in_=xhi[:, 4 * jg : 4 * jg + 4, hsl])
                xl = xl_pool.tile([HBS, 4, tlen], dt.bfloat16,
                                  name=f"xl{h}_{jg}", tag="xl")
                nc.sync.dma_start(out=xl[:], in_=xlo[:, 4 * jg : 4 * jg + 4, hsl])
                for dj in range(4):
                    j = 4 * jg + dj
                    for (o, w, acc) in lchunks:
                        nc.tensor.matmul(
                            acc[:], lhsT=laE_sb[:, j, :],
                            rhs=xh[:, dj, o : o + w],
                            start=(j == 0), stop=(j == NJ - 1),
                        )
                xq_tiles = []
                for ts in range(ts_n):
                    tsl = slice(ts * 128, (ts + 1) * 128)
                    bank = rot_ps.tile([128, 512], dt.float32,
                                       name=f"bank{h}_{jg}_{ts}", tag="bank")
                    for dj in range(4):
                        j = 4 * jg + dj
                        nc.tensor.matmul(
                            bank[:, dj * HBS : (dj + 1) * HBS],
                            lhsT=xh[:, dj, tsl], rhs=Hs_sb[:, j, :],
                            start=(dj == 0), stop=False,
                        )
                        nc.tensor.matmul(
                            bank[:, dj * HBS : (dj + 1) * HBS],
                            lhsT=xl[:, dj, tsl], rhs=Hs_sb[:, j, :],
                            start=False, stop=(dj == 3),
                        )
                    nb = 512 // QB
                    amax = qsm.tile([128, nb], dt.float32, name=f"am{h}{jg}{ts}", tag="amax")
                    nc.vector.tensor_reduce(
                        out=amax[:], in_=bank[:].rearrange("p (b s) -> p b s", s=QB),
                        axis=mybir.AxisListType.X, op=Alu.max,
                        apply_absolute_value=True,
                    )
                    ra = qsm.tile([128, nb], dt.float32, name=f"ra{h}{jg}{ts}", tag="ra")
                    nc.vector.reciprocal(out=ra[:], in_=amax[:])
                    rs6 = qsm.tile([128, nb], dt.float32, name=f"rs6{h}{jg}{ts}", tag="rs6")
                    nc.scalar.mul(out=rs6[:], in_=ra[:], mul=6.0)
                    sc = qsm.tile([128, nb], dt.float32, name=f"sc{h}{jg}{ts}", tag="sc")
                    nc.scalar.mul(out=sc[:], in_=amax[:], mul=1.0 / 6.0)
                    z = z_pool.tile([128, 512], dt.float32, name=f"z{h}{jg}{ts}", tag="z")
                    nc.vector.tensor_tensor(
                        out=z[:].rearrange("p (b s) -> p b s", s=QB),
                        in0=bank[:].rearrange("p (b s) -> p b s", s=QB),
                        in1=rs6[:].unsqueeze(2).broadcast_to([128, nb, QB]),
                        op=Alu.mult,
                    )
                    r23 = r23_pool.tile([128, 512], dt.float32, name=f"r23{h}{jg}{ts}", tag="r23")
                    nc.vector._custom_dve(
                        SNAP_A, out=r23[:], in0=z[:], s0=TH23, s1=C_INT, imm2=C_EVEN,
                    )
                    f = f_pool.tile([128, 512], dt.float32, name=f"f{h}{jg}{ts}", tag="f")
                    nc.vector._custom_dve(
                        SNAP_B, out=f[:], in0=z[:], in1=r23[:], s0=THF, s1=C_HALF,
                    )
                    xq_t = xq_pool.tile([128, 512], dt.bfloat16, name=f"xq{h}{jg}{ts}", tag="xq")
                    nc.gpsimd.tensor_tensor(
                        out=xq_t[:].rearrange("p (b s) -> p b s", s=QB),
                        in0=f[:].rearrange("p (b s) -> p b s", s=QB),
                        in1=sc[:].unsqueeze(2).broadcast_to([128, nb, QB]),
                        op=Alu.mult,
                    )
                    xq_tiles.append(xq_t)
                for dj in range(4):
                    j = 4 * jg + dj
                    pt = tr_ps.tile([128, tlen], dt.bfloat16, name=f"pt{h}_{j}", tag="pt")
                    for ts in range(ts_n):
                        nc.tensor.matmul(
                            pt[:, ts * 128 : (ts + 1) * 128],
                            lhsT=xq_tiles[ts][:, dj * HBS : (dj + 1) * HBS],
                            rhs=ident_bf[:], is_transpose=True,
                            start=(ts == 0), stop=(ts == ts_n - 1),
                        )
                    nc.scalar.copy(out=xqT[:, j, hsl], in_=pt[:])
            for (o, w, acc) in lchunks:
                nc.scalar.copy(out=t1T[:RANK, t0 + o : t0 + o + w], in_=acc[:])

        def emit_p2(h, t0, tlen, wtiles):
            ts_n = tlen // 128
            for og in range(OG_N):
                wt = wtiles[og]
                for th in range(ts_n):
                    tsl = slice(t0 + th * 128, t0 + (th + 1) * 128)
                    po = out_ps.tile([128, 512], dt.float32,
                                     name=f"po{h}{og}{th}", tag="po")
                    for k in range(NJ):
                        nc.tensor.matmul(
                            po[:], lhsT=xqT[:, k, tsl], rhs=wt[:, k, :],
                            start=(k == 0), stop=False,
                        )
                    nc.tensor.matmul(
                        po[:], lhsT=t1T[:, tsl], rhs=lbA_sb[:, og, :],
                        start=False, stop=True,
                    )
                    ot = out_pool.tile([128, 512], dt.float32,
                                       name=f"ot{h}{og}{th}", tag="ot")
                    nc.scalar.copy(out=ot[:], in_=po[:])
                    nc.sync.dma_start(
                        out=y[tsl, og * 512 : (og + 1) * 512], in_=ot[:]
                    )

        w0 = emit_w_loads(0)
        emit_p1(0, 0, H0)
        w1 = emit_w_loads(1)
        emit_p1(1, H0, NT - H0)
        emit_p2(0, 0, H0, w0)
        emit_p2(1, H0, NT - H0, w1)


_NC_CACHE = None


def _get_nc():
    global _NC_CACHE
    if _NC_CACHE is None:
        _NC_CACHE = _build_nc()
    return _NC_CACHE


# ---------------- host wrapper ----------------
def _prep_inputs(x, S_in, H_block, w_quantized, lora_a, lora_b, bias):
    import ml_dtypes
    BF16 = ml_dtypes.bfloat16
    x = np.asarray(x, dtype=F32)
    S_in = np.asarray(S_in, dtype=F32)
    H_block = np.asarray(H_block, dtype=F32)
    w_quantized = np.asarray(w_quantized, dtype=F32)
    lora_a = np.asarray(lora_a, dtype=F32)
    lora_b = np.asarray(lora_b, dtype=F32)
    bias = np.asarray(bias, dtype=F32)

    x_flat = x.reshape(NTOK, D_IN)
    x_hi = x_flat.astype(BF16)
    x_lo = (x_flat - x_hi.astype(F32)).astype(BF16)

    Ssq = S_in.reshape(NJ, HBS).T                        # [c, j]
    Hsign = np.sign(H_block).astype(F32)                 # +/-1 exact
    Hs = (Ssq[:, :, None] * Hsign[:, None, :]).astype(BF16)  # [c, j, c']

    la3 = lora_a.reshape(RANK, NJ, HBS)                  # [r, j, c']
    la_eff = np.einsum(
        "cd,rjd->cjr", H_block.astype(np.float64), la3.astype(np.float64)
    )
    laE = (Ssq[:, :, None] * la_eff.astype(F32)).astype(BF16)  # [c, j, r]

    rinv = np.float64(1.0) / np.sqrt(np.float64(HBS))
    # w4[c, og, j, o] = w[og*512+o, j*128+c] / sqrt(128)
    w4 = np.ascontiguousarray(
        (w_quantized.astype(np.float64) * rinv)
        .astype(F32)
        .reshape(OG_N, 512, NJ, HBS)
        .transpose(3, 0, 2, 1)
        .astype(BF16)
    )
    lbA = np.concatenate(
        [lora_b.T, bias.reshape(1, D_OUT)], axis=0
    ).reshape(RANK + 1, OG_N, 512).astype(BF16)
    lbA = np.ascontiguousarray(lbA)

    per_core = []
    for c in range(NCORES):
        tsl = slice(c * NT, (c + 1) * NT)
        # [c, j, t] feature-major per-core activations
        xh5 = np.ascontiguousarray(
            x_hi[tsl].reshape(NT, NJ, HBS).transpose(2, 1, 0)
        )
        xl5 = np.ascontiguousarray(
            x_lo[tsl].reshape(NT, NJ, HBS).transpose(2, 1, 0)
        )
        per_core.append(
            {"xhi": xh5, "xlo": xl5, "Hs": Hs, "laE": laE, "w4": w4, "lbA": lbA}
        )
    return per_core


def kernel(x, S_in, H_block, w_quantized, lora_a, lora_b, bias):
    in_maps = _prep_inputs(x, S_in, H_block, w_quantized, lora_a, lora_b, bias)
    nc = _get_nc()
    res = run_bass_kernel_spmd(nc, in_maps, core_ids=list(range(NCORES)))
    out = np.concatenate([res.results[c]["y"] for c in range(NCORES)], axis=0)
    return out.reshape(B, S, D_OUT).astype(F32)


# revision 15
# speedup vs baseline: 1.1201x; 1.0405x over previous
"""CascadeHadamardLinear Trainium2 kernel (8-core data-parallel over tokens).

Math per token row x[4096]:
  x_rot = (x * S_in) @ blockdiag(H_128)     H = sign/sqrt(128) Hadamard
  x_q   = NVFP4 fake-quant of x_rot (16-elem blocks, e2m1 snap, RNE)
  out   = x_q @ W^T + (x_rot @ la^T) @ lb^T + bias

Device computes with x_rot~ = sqrt(128)*x_rot via an EXACT +/-1 sign
matrix (S folded in) in bf16, with x split host-side into x_hi + x_lo
(dual bf16) for fp32-class rotation at bf16 matmul rate. The quantizer
is scale-invariant (z = 6*v/amax), so f is unchanged; sqrt(128) is
folded out of the weights on the host (w~ = w/sqrt(128)).

Per core (1024 tokens, halves of 256/768 for phase overlap):
  p1(h): per jg (4 hadamard blocks): DMA x_hi/x_lo; LoRA1 t1 += la_eff^T
    @ x_hi (la_eff host-precomputed, includes S, H, 1/sqrt(128)); per
    128-token tile: 8 bf16 rotation MMs -> PSUM bank [128,512]; quant:
    absmax-16 (DVE), reciprocal (DVE), rs6/sc (ACT muls), fused
    z*rs6 + 3-way e2m1 magic-add snap in ONE custom DVE op, xq = f*sc
    -> bf16 (GPSIMD); PE-transpose xq -> xqT[d-major].
  p2(h): per og (512 out cols): stream w~ (bf16, host layout
    [c,og,j,o] = 32KB contiguous lines); per 128-token tile: PSUM
    accumulation of 32 main MMs + 1 merged (LoRA2+bias) K=33 MM
    (t1T has a ones row, lbT an appended bias row); ACT evac, DMA out.
Emission p1(0), w-prefetch, p2(0), p1(1), p2(1): Tile backfills PE
idle in DVE-bound p1 windows with ready p2 matmuls.
"""

import os
import sys

for _p in ("/opt/trn_rl_repo",):
    if os.path.isdir(_p) and _p not in sys.path:
        sys.path.insert(0, _p)

import numpy as np

import concourse.bass as bass
import concourse.mybir as mybir
import concourse.tile as tile
from concourse import bacc
from concourse.bass_utils import run_bass_kernel_spmd

F32 = np.float32

# ---------------- problem constants (hardcoded per contract) ----------------
B, S, D_IN, D_OUT, RANK, HBS = 4, 2048, 4096, 4096, 32, 128
NTOK = B * S                  # 8192
NCORES = 8
NT = NTOK // NCORES           # 1024 tokens per core
NJ = D_IN // HBS              # 32 hadamard blocks
QB = 16                       # quant block size
OG_N = D_OUT // 512           # 8 output column groups
H0 = 256                      # prologue half (tokens)

# quant snap constants (1.5*2^k magic so ulp is uniform on both sides of c)
C_HALF = 6291456.0            # 1.5*2^22, ulp 0.5
C_INT = 12582912.0            # 1.5*2^23, ulp 1.0
C_EVEN = 25165824.0           # 1.5*2^24, ulp 2.0
THF = 5.0625                  # 2.25^2
TH23 = 20.25                  # 4.5^2


# ---------------- custom DVE ops (e2m1 level snap, 2 passes, no t2) --------
def _register_snap_ops():
    from concourse.dve_spec import (
        Spec, Src0, Src1, C0, C1, C2, lower as dve_lower, sq, select, _has_src1,
    )
    from concourse.dve_ops import (
        DveOp, OPS, CUSTOM_DVE_SPECS, _SUB_OPCODE_FOR_NAME, _CUSTOM_DVE_ROW_BASE,
    )
    from concourse.dve_uop import DveOpSpec
    from concourse.dve_table_gen import dve_ver_for

    def _ref_a(in0, in1, c0, c1, c2):
        z = in0.astype(F32)
        r_int = (z + F32(c1)) - F32(c1)
        r_even = (z + F32(c2)) - F32(c2)
        return np.where(z * z < F32(c0), r_int, r_even).astype(F32)

    def _ref_b(in0, in1, c0, c1, c2):
        z = in0.astype(F32)
        r_half = (z + F32(c1)) - F32(c1)
        return np.where(z * z < F32(c0), r_half, in1.astype(F32)).astype(F32)

    def _mk(name, body, ref):
        if name in _SUB_OPCODE_FOR_NAME:
            return next(op for op in OPS if op.name == name)
        spec = Spec(body=body, reference=ref)
        row = _CUSTOM_DVE_ROW_BASE + len(OPS)
        assert row < 0x20
        ver = dve_ver_for("TRN2")
        uops = dve_lower(spec, ver=ver)
        sha = DveOpSpec(
            name=name, opcode=row, uops=uops, rd1_en=_has_src1(spec)
        ).sha(ver)
        op = DveOp(name, spec, subdim=False, uops_sha={ver: sha})
        OPS.append(op)
        CUSTOM_DVE_SPECS[name] = spec
        _SUB_OPCODE_FOR_NAME[name] = row
        return op

    z = Src0
    snap_a = _mk(
        "SNAP_A_ANT",
        select(sq(z) < C0, (z + C1) - C1, (z + C2) - C2),
        _ref_a,
    )
    snap_b = _mk(
        "SNAP_B_ANT",
        select(sq(z) < C0, (z + C1) - C1, Src1),
        _ref_b,
    )
    return snap_a, snap_b


SNAP_A, SNAP_B = _register_snap_ops()


# ---------------- device kernel ----------------
def _build_nc():
    nc = bacc.Bacc(
        "TRN2", target_bir_lowering=False, debug=False, num_devices=NCORES
    )
    dt = mybir.dt
    xhi = nc.dram_tensor("xhi", [HBS, NJ, NT], dt.bfloat16, kind="ExternalInput")
    xlo = nc.dram_tensor("xlo", [HBS, NJ, NT], dt.bfloat16, kind="ExternalInput")
    Hs = nc.dram_tensor("Hs", [HBS, NJ, HBS], dt.bfloat16, kind="ExternalInput")
    laE = nc.dram_tensor("laE", [HBS, NJ, RANK], dt.bfloat16, kind="ExternalInput")
    w4 = nc.dram_tensor("w4", [HBS, OG_N, NJ, 512], dt.bfloat16, kind="ExternalInput")
    lbA = nc.dram_tensor("lbA", [RANK + 1, OG_N, 512], dt.bfloat16, kind="ExternalInput")
    y = nc.dram_tensor("y", [NT, D_OUT], dt.float32, kind="ExternalOutput")

    with tile.TileContext(nc) as tc:
        _emit(nc, tc, xhi, xlo, Hs, laE, w4, lbA, y)
    nc.compile()
    return nc


def _emit(nc, tc, xhi, xlo, Hs, laE, w4, lbA, y):
    from contextlib import ExitStack

    dt = mybir.dt
    Alu = mybir.AluOpType
    Act = mybir.ActivationFunctionType

    with ExitStack() as ctx:
        consts = ctx.enter_context(tc.tile_pool(name="consts", bufs=1))

        Hs_sb = consts.tile([HBS, NJ, HBS], dt.bfloat16)
        nc.sync.dma_start(out=Hs_sb[:], in_=Hs[:])
        laE_sb = consts.tile([HBS, NJ, RANK], dt.bfloat16)
        nc.sync.dma_start(out=laE_sb[:], in_=laE[:])
        lbA_sb = consts.tile([RANK + 1, OG_N, 512], dt.bfloat16)
        ident_bf = consts.tile([128, 128], dt.bfloat16)
        from concourse.masks import make_identity
        make_identity(nc, ident_bf[:])

        t1T = consts.tile([RANK + 1, NT], dt.bfloat16)
        nc.vector.memset(t1T[RANK : RANK + 1, :], 1.0)

        # xqT[c, j, t] : feature-major quantized activations (bf16)
        xqT = consts.tile([HBS, NJ, NT], dt.bfloat16)

        # working pools
        xh_pool = ctx.enter_context(tc.tile_pool(name="xh", bufs=2))
        xl_pool = ctx.enter_context(tc.tile_pool(name="xl", bufs=2))
        qsm = ctx.enter_context(tc.tile_pool(name="qsm", bufs=4))
        z_pool = ctx.enter_context(tc.tile_pool(name="z", bufs=3))
        r23_pool = ctx.enter_context(tc.tile_pool(name="r23", bufs=2))
        f_pool = ctx.enter_context(tc.tile_pool(name="f", bufs=3))
        xq_pool = ctx.enter_context(tc.tile_pool(name="xq", bufs=8))
        wbf_pool = ctx.enter_context(tc.tile_pool(name="wbf", bufs=2))
        out_pool = ctx.enter_context(tc.tile_pool(name="out", bufs=3))
        rot_ps = ctx.enter_context(tc.tile_pool(name="rotps", bufs=2, space="PSUM"))
        tr_ps = ctx.enter_context(tc.tile_pool(name="trps", bufs=2, space="PSUM"))
        t1_ps = ctx.enter_context(tc.tile_pool(name="t1ps", bufs=2, space="PSUM"))
        out_ps = ctx.enter_context(tc.tile_pool(name="outps", bufs=2, space="PSUM"))

        def emit_w_loads(h):
            tiles = []
            for og in range(OG_N):
                wt = wbf_pool.tile([HBS, NJ, 512], dt.bfloat16,
                                   name=f"w{h}_{og}", tag="wbf")
                nc.sync.dma_start(out=wt[:], in_=w4[:, og])
                tiles.append(wt)
            return tiles

        def emit_p1(h, t0, tlen):
            hsl = slice(t0, t0 + tlen)
            ts_n = tlen // 128
            # LoRA1 accumulators in <=512-token chunks (one PSUM bank each)
            lchunks = []
            o = 0
            while o < tlen:
                w = min(512, tlen - o)
                acc = t1_ps.tile([RANK, w], dt.float32,
                                 name=f"t1acc{h}_{o}", tag="t1acc")
                lchunks.append((o, w, acc))
                o += w
            for jg in range(NJ // 4):
                xh = xh_pool.tile([HBS, 4, tlen], dt.bfloat16,
                                  name=f"xh{h}_{jg}", tag="xh")
                nc.sync.dma_start(out=xh[:], in_=xhi[:, 4 * jg : 4 * jg + 4, hsl])
                xl = xl_pool.tile([HBS, 4, tlen], dt.bfloat16,
                                  name=f"xl{h}_{jg}", tag="xl")
                nc.sync.dma_start(out=xl[:], in_=xlo[:, 4 * jg : 4 * jg + 4, hsl])
                for dj in range(4):
                    j = 4 * jg + dj
                    for (o, w, acc) in lchunks:
                        nc.tensor.matmul(
                            acc[:], lhsT=laE_sb[:, j, :],
                            rhs=xh[:, dj, o : o + w],
                            start=(j == 0), stop=(j == NJ - 1),
                        )
                xq_tiles = []
                for ts in range(ts_n):
                    tsl = slice(ts * 128, (ts + 1) * 128)
                    bank = rot_ps.tile([128, 512], dt.float32,
                                       name=f"bank{h}_{jg}_{ts}", tag="bank")
                    for dj in range(4):
                        j = 4 * jg + dj
                        nc.tensor.matmul(
                            bank[:, dj * HBS : (dj + 1) * HBS],
                            lhsT=xh[:, dj, tsl], rhs=Hs_sb[:, j, :],
                            start=(dj == 0), stop=False,
                        )
                        nc.tensor.matmul(
                            bank[:, dj * HBS : (dj + 1) * HBS],
                            lhsT=xl[:, dj, tsl], rhs=Hs_sb[:, j, :],
                            start=False, stop=(dj == 3),
                        )
                    nb = 512 // QB
                    amax = qsm.tile([128, nb], dt.float32, name=f"am{h}{jg}{ts}", tag="amax")
                    nc.vector.tensor_reduce(
                        out=amax[:], in_=bank[:].rearrange("p (b s) -> p b s", s=QB),
                        axis=mybir.AxisListType.X, op=Alu.max,
                        apply_absolute_value=True,
                    )
                    ra = qsm.tile([128, nb], dt.float32, name=f"ra{h}{jg}{ts}", tag="ra")
                    nc.vector.reciprocal(out=ra[:], in_=amax[:])
                    rs6 = qsm.tile([128, nb], dt.float32, name=f"rs6{h}{jg}{ts}", tag="rs6")
                    nc.scalar.mul(out=rs6[:], in_=ra[:], mul=6.0)
                    sc = qsm.tile([128, nb], dt.float32, name=f"sc{h}{jg}{ts}", tag="sc")
                    nc.scalar.mul(out=sc[:], in_=amax[:], mul=1.0 / 6.0)
                    z = z_pool.tile([128, 512], dt.float32, name=f"z{h}{jg}{ts}", tag="z")
                    nc.vector.tensor_tensor(
                        out=z[:].rearrange("p (b s) -> p b s", s=QB),
                        in0=bank[:].rearrange("p (b s) -> p b s", s=QB),
                        in1=rs6[:].unsqueeze(2).broadcast_to([128, nb, QB]),
                        op=Alu.mult,
                    )
                    r23 = r23_pool.tile([128, 512], dt.float32, name=f"r23{h}{jg}{ts}", tag="r23")
                    nc.vector._custom_dve(
                        SNAP_A, out=r23[:], in0=z[:], s0=TH23, s1=C_INT, imm2=C_EVEN,
                    )
                    f = f_pool.tile([128, 512], dt.float32, name=f"f{h}{jg}{ts}", tag="f")
                    nc.vector._custom_dve(
                        SNAP_B, out=f[:], in0=z[:], in1=r23[:], s0=THF, s1=C_HALF,
                    )
                    xq_t = xq_pool.tile([128, 512], dt.bfloat16, name=f"xq{h}{jg}{ts}", tag="xq")
                    nc.gpsimd.tensor_tensor(
                        out=xq_t[:].rearrange("p (b s) -> p b s", s=QB),
                        in0=f[:].rearrange("p (b s) -> p b s", s=QB),
                        in1=sc[:].unsqueeze(2).broadcast_to([128, nb, QB]),
                        op=Alu.mult,
                    )
                    xq_tiles.append(xq_t)
                for dj in range(4):
                    j = 4 * jg + dj
                    pt = tr_ps.tile([128, tlen], dt.bfloat16, name=f"pt{h}_{j}", tag="pt")
                    for ts in range(ts_n):
                        nc.tensor.matmul(
                            pt[:, ts * 128 : (ts + 1) * 128],
                            lhsT=xq_tiles[ts][:, dj * HBS : (dj + 1) * HBS],
                            rhs=ident_bf[:], is_transpose=True,
                            start=(ts == 0), stop=(ts == ts_n - 1),
                        )
                    nc.scalar.copy(out=xqT[:, j, hsl], in_=pt[:])
            for (o, w, acc) in lchunks:
                nc.scalar.copy(out=t1T[:RANK, t0 + o : t0 + o + w], in_=acc[:])

        def emit_p2(h, t0, tlen, wtiles):
            ts_n = tlen // 128
            for og in range(OG_N):
                wt = wtiles[og]
                for th in range(ts_n):
                    tsl = slice(t0 + th * 128, t0 + (th + 1) * 128)
                    po = out_ps.tile([128, 512], dt.float32,
                                     name=f"po{h}{og}{th}", tag="po")
                    for k in range(NJ):
                        nc.tensor.matmul(
                            po[:], lhsT=xqT[:, k, tsl], rhs=wt[:, k, :],
                            start=(k == 0), stop=False,
                        )
                    nc.tensor.matmul(
                        po[:], lhsT=t1T[:, tsl], rhs=lbA_sb[:, og, :],
                        start=False, stop=True,
                    )
                    ot = out_pool.tile([128, 512], dt.float32,
                                       name=f"ot{h}{og}{th}", tag="ot")
                    nc.scalar.copy(out=ot[:], in_=po[:])
                    nc.sync.dma_start(
                        out=y[tsl, og * 512 : (og + 1) * 512], in_=ot[:]
                    )

        emit_p1(0, 0, H0)
        w0 = emit_w_loads(0)
        nc.sync.dma_start(out=lbA_sb[:], in_=lbA[:])
        emit_p1(1, H0, NT - H0)
        w1 = emit_w_loads(1)
        emit_p2(0, 0, H0, w0)
        emit_p2(1, H0, NT - H0, w1)


_NC_CACHE = None


def _get_nc():
    global _NC_CACHE
    if _NC_CACHE is None:
        _NC_CACHE = _build_nc()
    return _NC_CACHE


# ---------------- host wrapper ----------------
def _prep_inputs(x, S_in, H_block, w_quantized, lora_a, lora_b, bias):
    import ml_dtypes
    BF16 = ml_dtypes.bfloat16
    x = np.asarray(x, dtype=F32)
    S_in = np.asarray(S_in, dtype=F32)
    H_block = np.asarray(H_block, dtype=F32)
    w_quantized = np.asarray(w_quantized, dtype=F32)
    lora_a = np.asarray(lora_a, dtype=F32)
    lora_b = np.asarray(lora_b, dtype=F32)
    bias = np.asarray(bias, dtype=F32)

    x_flat = x.reshape(NTOK, D_IN)
    x_hi = x_flat.astype(BF16)
    x_lo = (x_flat - x_hi.astype(F32)).astype(BF16)

    Ssq = S_in.reshape(NJ, HBS).T                        # [c, j]
    Hsign = np.sign(H_block).astype(F32)                 # +/-1 exact
    Hs = (Ssq[:, :, None] * Hsign[:, None, :]).astype(BF16)  # [c, j, c']

    la3 = lora_a.reshape(RANK, NJ, HBS)                  # [r, j, c']
    la_eff = np.einsum(
        "cd,rjd->cjr", H_block.astype(np.float64), la3.astype(np.float64)
    )
    laE = (Ssq[:, :, None] * la_eff.astype(F32)).astype(BF16)  # [c, j, r]

    rinv = np.float64(1.0) / np.sqrt(np.float64(HBS))
    # w4[c, og, j, o] = w[og*512+o, j*128+c] / sqrt(128)
    w4 = np.ascontiguousarray(
        (w_quantized.astype(np.float64) * rinv)
        .astype(F32)
        .reshape(OG_N, 512, NJ, HBS)
        .transpose(3, 0, 2, 1)
        .astype(BF16)
    )
    lbA = np.concatenate(
        [lora_b.T, bias.reshape(1, D_OUT)], axis=0
    ).reshape(RANK + 1, OG_N, 512).astype(BF16)
    lbA = np.ascontiguousarray(lbA)

    per_core = []
    for c in range(NCORES):
        tsl = slice(c * NT, (c + 1) * NT)
        # [c, j, t] feature-major per-core activations
        xh5 = np.ascontiguousarray(
            x_hi[tsl].reshape(NT, NJ, HBS).transpose(2, 1, 0)
        )
        xl5 = np.ascontiguousarray(
            x_lo[tsl].reshape(NT, NJ, HBS).transpose(2, 1, 0)
        )
        per_core.append(
            {"xhi": xh5, "xlo": xl5, "Hs": Hs, "laE": laE, "w4": w4, "lbA": lbA}
        )
    return per_core


def kernel(x, S_in, H_block, w_quantized, lora_a, lora_b, bias):
    in_maps = _prep_inputs(x, S_in, H_block, w_quantized, lora_a, lora_b, bias)
    nc = _get_nc()
    res = run_bass_kernel_spmd(nc, in_maps, core_ids=list(range(NCORES)))
    out = np.concatenate([res.results[c]["y"] for c in range(NCORES)], axis=0)
    return out.reshape(B, S, D_OUT).astype(F32)
